# revision 5
# baseline (speedup 1.0000x reference)
"""EGAT (edge-featured GAT) Trainium2 Bass kernel, 8-core SPMD.

Strategy: 1D node partition. Each core owns a 256-row slab of the N=2048
nodes. All [P,N,N] attention tensors live in SBUF transposed ([j, (p,i)]
layout, partition = neighbor j) so the attention*V contraction over j maps
directly onto the PE array. Attention state never touches DRAM between the
5 layers. The only cross-core exchange is an AllGather of the final layer's
Wh_out ([2048,17] incl. a ones column used to get softmax row sums for free
from the matmul).

Host side: Wh/f_src/f_dst for heads 1-4 depend only on inputs -> numpy.
Final elu+log_softmax on [2048,16] logits -> numpy.
"""

import sys
import os

sys.path.insert(0, "/opt/trn_rl_repo")

import numpy as np

import concourse.bass as bass
import concourse.tile as tile
from concourse import mybir
from concourse.bass_utils import run_bass_kernel_spmd
from concourse.masks import make_identity

# problem constants (hardcoded per contract)
N = 2048
P = 4
FIN = 256
FH = 64
H = 4
C = 16
ALPHA = 0.2
NCORES = 8
ISLAB = N // NCORES          # 256 rows per core
NJC = N // 128               # 16 j-chunks of 128 partitions
PI = P * ISLAB               # 1024 free elements per (p,i) tile

FP32 = mybir.dt.float32
BF16 = mybir.dt.bfloat16

TRACE = False                # test.py flips this for profiling
_LAST = {}                   # exec stats for test.py


def _bcast_ap(src_ap, nparts):
    """Partition-broadcast a [1, F] DRAM AP to [nparts, F]."""
    return bass.AP(
        tensor=src_ap.tensor,
        offset=src_ap.offset,
        ap=[[0, nparts]] + [list(d) for d in src_ap.ap[-1:]],
    )


def _split_multi_waits(nc):
    """walrus in this env accepts one sync-wait per compute instruction;
    split extras onto same-engine NoOps placed just before."""
    n = 0
    for fn in nc.m.functions:
        for bb in fn.blocks:
            new_list = []
            for inst in bb.instructions:
                si = inst.sync_info
                if si and si.on_wait and len(si.on_wait) > 1:
                    waits = list(si.on_wait)
                    for w in waits[:-1]:
                        new_list.append(
                            mybir.InstNoOp(
                                name=f"{inst.name}-wsplit{n}",
                                engine=inst.engine,
                                sync_info=mybir.SyncInfo(on_wait=[w], on_update=[]),
                            )
                        )
                        n += 1
                    inst.sync_info = mybir.SyncInfo(
                        on_wait=[waits[-1]], on_update=list(si.on_update or [])
                    )
                new_list.append(inst)
            bb.instructions = new_list
    return n


def _build_nc():
    nc = bass.Bass(num_devices=NCORES)

    ea_p = nc.declare_dram_parameter("ea", [N, PI], FP32, isOutput=False)
    fsrc_p = nc.declare_dram_parameter("fsrc", [H, ISLAB], FP32, isOutput=False)
    fdst_p = nc.declare_dram_parameter("fdst", [128, H * NJC], FP32, isOutput=False)
    whaug_p = nc.declare_dram_parameter("whaug", [H, NJC, 128, FH + 1], BF16, isOutput=False)
    wout_p = nc.declare_dram_parameter("wout", [8, 128, C], BF16, isOutput=False)
    asrc_p = nc.declare_dram_parameter("asrc", [C, 1], FP32, isOutput=False)
    adst_p = nc.declare_dram_parameter("adst", [1, C], FP32, isOutput=False)
    out_p = nc.declare_dram_parameter("out", [C, ISLAB], FP32, isOutput=True)

    Act = mybir.ActivationFunctionType
    Alu = mybir.AluOpType

    with tile.TileContext(nc) as tc:
        import contextlib
        with contextlib.ExitStack() as ctx:
            singles = ctx.enter_context(tc.tile_pool(name="singles", bufs=1))
            dram = ctx.enter_context(tc.tile_pool(name="dram", bufs=1, space="DRAM"))
            fsrcbc_pool = ctx.enter_context(tc.tile_pool(name="fsrcbc", bufs=2))
            whaug_pool = ctx.enter_context(tc.tile_pool(name="whaug", bufs=2))
            ea_pool = ctx.enter_context(tc.tile_pool(name="ea", bufs=3))
            e_pool = ctx.enter_context(tc.tile_pool(name="e", bufs=3))
            eh_pool = ctx.enter_context(tc.tile_pool(name="eh", bufs=2))
            sc_pool = ctx.enter_context(tc.tile_pool(name="sc", bufs=2))
            u_pool = ctx.enter_context(tc.tile_pool(name="u", bufs=2))
            rrow_pool = ctx.enter_context(tc.tile_pool(name="rrow", bufs=2))
            rbc_pool = ctx.enter_context(tc.tile_pool(name="rbc", bufs=2))
            post_pool = ctx.enter_context(tc.tile_pool(name="post", bufs=4))
            av_psum = ctx.enter_context(tc.tile_pool(name="av", bufs=1, space="PSUM"))
            aux_psum = ctx.enter_context(tc.tile_pool(name="aux", bufs=1, space="PSUM"))
            tp_psum = ctx.enter_context(tc.tile_pool(name="tp", bufs=2, space="PSUM"))

            # ---- singles ----
            fdst_sb = singles.tile([128, H * NJC], FP32)
            nc.sync.dma_start(out=fdst_sb, in_=fdst_p[:, :])
            asrc_sb = singles.tile([C, 1], FP32)
            nc.sync.dma_start(out=asrc_sb, in_=asrc_p[:, :])
            adst_bc = singles.tile([128, C], FP32)
            nc.sync.dma_start(out=adst_bc, in_=_bcast_ap(adst_p[0:1, :], 128))
            identity = singles.tile([128, 128], FP32)
            make_identity(nc, identity)
            wout_sb = []
            for c8 in range(8):
                w = singles.tile([128, C], BF16, tag=f"wout{c8}", name=f"wout{c8}")
                nc.sync.dma_start(out=w, in_=wout_p[c8, :, :])
                wout_sb.append(w)
            xcatT = []
            for c8 in range(8):
                x = singles.tile([128, ISLAB], BF16, tag=f"xcat{c8}", name=f"xcat{c8}")
                xcatT.append(x)

            u_prev = [None] * NJC
            recip_bc_prev = None

            # ---------------- heads 1..4 ----------------
            for h in range(H):
                fsrc_bc = fsrcbc_pool.tile([128, ISLAB], FP32, tag="fsrcbc")
                nc.sync.dma_start(out=fsrc_bc, in_=_bcast_ap(fsrc_p[h : h + 1, :], 128))
                whaug_sb = []
                for jc in range(NJC):
                    w = whaug_pool.tile([128, FH + 1], BF16, tag=f"whaug{jc}", name=f"whaug{jc}")
                    nc.sync.dma_start(out=w, in_=whaug_p[h, jc, :, :])
                    whaug_sb.append(w)

                av = [av_psum.tile([FH + 1, ISLAB], FP32, tag=f"av{p}", name=f"av{p}") for p in range(P)]

                u_cur = [None] * NJC
                for jc in range(NJC):
                    e_t = e_pool.tile([128, ISLAB], FP32, tag="e")
                    idx = h * NJC + jc
                    nc.scalar.activation(
                        e_t, fsrc_bc, Act.Prelu,
                        bias=fdst_sb[:, idx : idx + 1], alpha=ALPHA,
                    )
                    sc_t = sc_pool.tile([128, PI], FP32, tag="sc")
                    if h == 0:
                        ea_t = ea_pool.tile([128, PI], FP32, tag="ea")
                        nc.sync.dma_start(
                            out=ea_t, in_=ea_p[jc * 128 : (jc + 1) * 128, :]
                        )
                        for p in range(P):
                            sl = slice(p * ISLAB, (p + 1) * ISLAB)
                            nc.vector.tensor_mul(sc_t[:, sl], e_t, ea_t[:, sl])
                    else:
                        eh_t = eh_pool.tile([128, PI], FP32, tag="eh")
                        for p in range(P):
                            sl = slice(p * ISLAB, (p + 1) * ISLAB)
                            nc.vector.tensor_mul(eh_t[:, sl], e_t, recip_bc_prev[:, sl])
                        for p in range(P):
                            sl = slice(p * ISLAB, (p + 1) * ISLAB)
                            nc.vector.tensor_mul(sc_t[:, sl], eh_t[:, sl], u_prev[jc][:, sl])
                    u_t = u_pool.tile([128, PI], BF16, tag=f"u{jc}")
                    nc.scalar.activation(u_t, sc_t, Act.Exp)
                    u_cur[jc] = u_t
                    for p in range(P):
                        sl = slice(p * ISLAB, (p + 1) * ISLAB)
                        nc.tensor.matmul(
                            av[p][:, :], whaug_sb[jc], u_t[:, sl],
                            start=(jc == 0), stop=(jc == NJC - 1),
                        )

                # ---- layer post: recip of row sums, xcat = elu(h' / s) ----
                recip_row = rrow_pool.tile([1, PI], FP32, tag="rrow")
                for p in range(P):
                    sl = slice(p * ISLAB, (p + 1) * ISLAB)
                    nc.vector.reciprocal(recip_row[:, sl], av[p][FH : FH + 1, :])
                rb = dram.tile([1, PI], FP32, tag=f"rb{h}")
                nc.sync.dma_start(out=rb[:], in_=recip_row)
                recip_bc = rbc_pool.tile([128, PI], FP32, tag="rbc")
                nc.sync.dma_start(out=recip_bc, in_=_bcast_ap(rb[0:1, :], 128))

                for p in range(P):
                    sl = slice(p * ISLAB, (p + 1) * ISLAB)
                    xn = post_pool.tile([FH, ISLAB], FP32, tag="xn", bufs=2)
                    nc.vector.tensor_mul(xn, av[p][0:FH, :], recip_bc[0:FH, sl])
                    m = post_pool.tile([FH, ISLAB], FP32, tag="m", bufs=2)
                    nc.vector.tensor_scalar_min(m, xn, 0.0)
                    g = post_pool.tile([FH, ISLAB], FP32, tag="g", bufs=2)
                    nc.scalar.activation(g, m, Act.Exp)
                    g1 = post_pool.tile([FH, ISLAB], FP32, tag="g1", bufs=2)
                    nc.vector.tensor_scalar_add(g1, g, -1.0)
                    cidx = h * 2 + p // 2
                    r0 = (p % 2) * FH
                    nc.vector.tensor_max(xcatT[cidx][r0 : r0 + FH, :], xn, g1)

                u_prev = u_cur
                recip_bc_prev = recip_bc

            # ---------------- final layer prep ----------------
            wo_ps = aux_psum.tile([C, ISLAB], FP32, tag="wo")
            for c8 in range(8):
                nc.tensor.matmul(
                    wo_ps[:, :], wout_sb[c8], xcatT[c8],
                    start=(c8 == 0), stop=(c8 == 7),
                )
            whoutT_sb = singles.tile([C, ISLAB], FP32, tag="whoutT")
            nc.vector.tensor_copy(whoutT_sb, wo_ps)

            fs5_ps = aux_psum.tile([1, ISLAB], FP32, tag="fs5")
            nc.tensor.matmul(fs5_ps[:, :], asrc_sb, whoutT_sb, start=True, stop=True)
            fs5_row = singles.tile([1, ISLAB], FP32, tag="fs5row")
            nc.vector.tensor_copy(fs5_row, fs5_ps)
            fs5_d = dram.tile([1, ISLAB], FP32, tag="fs5d")
            nc.sync.dma_start(out=fs5_d[:], in_=fs5_row)
            fsrc5_bc = singles.tile([128, ISLAB], FP32, tag="fsrc5bc")
            nc.sync.dma_start(out=fsrc5_bc, in_=_bcast_ap(fs5_d[0:1, :], 128))

            # transpose Wh_outT -> [i, c] staging with ones column, allgather
            ag_in = dram.tile([ISLAB, C], FP32, tag="agin")
            for half in range(2):
                tp = tp_psum.tile([128, C], FP32, tag="tp")
                nc.tensor.transpose(
                    tp, whoutT_sb[:, half * 128 : (half + 1) * 128],
                    identity[0:C, 0:C],
                )
                st = post_pool.tile([128, C], FP32, tag="st", bufs=2)
                nc.vector.tensor_copy(st, tp)
                nc.gpsimd.dma_start(
                    out=ag_in[half * 128 : (half + 1) * 128, :], in_=st
                )
            ag_out = dram.tile([N, C], FP32, tag="agout")
            nc.gpsimd.collective_compute(
                "AllGather", Alu.bypass,
                replica_groups=[list(range(NCORES))],
                ins=[ag_in.opt()], outs=[ag_out.opt()],
            )
            lhsT5f = singles.tile([128, NJC, C], FP32, tag="lhsT5f")
            nc.gpsimd.dma_start(
                out=lhsT5f,
                in_=ag_out[:, :].rearrange("(jc jp) c -> jp jc c", jp=128),
            )
            lhsT5 = singles.tile([128, NJC, FH + 1], BF16, tag="lhsT5")
            nc.vector.memset(lhsT5, 0.0)
            nc.vector.tensor_copy(lhsT5[:, :, 0:C], lhsT5f)
            nc.vector.memset(lhsT5[:, :, FH : FH + 1], 1.0)
            fdst5_sb = singles.tile([128, NJC], FP32, tag="fdst5")
            for jc in range(NJC):
                tt = post_pool.tile([128, C], FP32, tag="f5t", bufs=2)
                nc.vector.tensor_mul(tt, lhsT5f[:, jc, 0:C], adst_bc)
                nc.vector.tensor_reduce(
                    fdst5_sb[:, jc : jc + 1], tt, axis=mybir.AxisListType.X, op=Alu.add
                )

            # ---------------- final layer ----------------
            av5 = [av_psum.tile([FH + 1, ISLAB], FP32, tag=f"av{p}", name=f"av5{p}") for p in range(P)]
            for jc in range(NJC):
                e_t = e_pool.tile([128, ISLAB], FP32, tag="e")
                nc.scalar.activation(
                    e_t, fsrc5_bc, Act.Prelu,
                    bias=fdst5_sb[:, jc : jc + 1], alpha=ALPHA,
                )
                eh_t = eh_pool.tile([128, PI], FP32, tag="eh")
                sc_t = sc_pool.tile([128, PI], FP32, tag="sc")
                for p in range(P):
                    sl = slice(p * ISLAB, (p + 1) * ISLAB)
                    nc.vector.tensor_mul(eh_t[:, sl], e_t, recip_bc_prev[:, sl])
                for p in range(P):
                    sl = slice(p * ISLAB, (p + 1) * ISLAB)
                    nc.vector.tensor_mul(sc_t[:, sl], eh_t[:, sl], u_prev[jc][:, sl])
                u_t = u_pool.tile([128, PI], BF16, tag=f"u{jc}")
                nc.scalar.activation(u_t, sc_t, Act.Exp)
                for p in range(P):
                    sl = slice(p * ISLAB, (p + 1) * ISLAB)
                    nc.tensor.matmul(
                        av5[p][:, :], lhsT5[:, jc, :], u_t[:, sl],
                        start=(jc == 0), stop=(jc == NJC - 1),
                    )

            r5 = rrow_pool.tile([1, PI], FP32, tag="rrow")
            for p in range(P):
                sl = slice(p * ISLAB, (p + 1) * ISLAB)
                nc.vector.reciprocal(r5[:, sl], av5[p][FH : FH + 1, :])
            r5s = rrow_pool.tile([1, PI], FP32, tag="r5s")
            nc.vector.tensor_scalar_mul(r5s, r5, 1.0 / P)
            r5d = dram.tile([1, PI], FP32, tag="r5d")
            nc.sync.dma_start(out=r5d[:], in_=r5s)
            r5bc = rbc_pool.tile([128, PI], FP32, tag="rbc")
            nc.sync.dma_start(out=r5bc, in_=_bcast_ap(r5d[0:1, :], 128))

            acc = None
            for p in range(P):
                sl = slice(p * ISLAB, (p + 1) * ISLAB)
                t5 = post_pool.tile([C, ISLAB], FP32, tag=f"t5_{p}", bufs=1, name=f"t5_{p}")
                nc.vector.tensor_mul(t5, av5[p][0:C, :], r5bc[0:C, sl])
                if acc is None:
                    acc = t5
                else:
                    a2 = post_pool.tile([C, ISLAB], FP32, tag=f"acc{p}", bufs=1, name=f"acc{p}")
                    nc.vector.tensor_add(a2, acc, t5)
                    acc = a2
            nc.sync.dma_start(out=out_p[:, :], in_=acc)

    _split_multi_waits(nc)
    return nc


_NC_CACHE = None


def _get_nc():
    global _NC_CACHE
    if _NC_CACHE is None:
        _NC_CACHE = _build_nc()
    return _NC_CACHE


def prepare_in_maps(x, edge_attr, W_heads, a_src_heads, a_dst_heads, W_out, a_src_out, a_dst_out):
    x = np.asarray(x, np.float32)
    edge_attr = np.asarray(edge_attr, np.float32)
    W_heads = np.asarray(W_heads, np.float32)
    a_src_heads = np.asarray(a_src_heads, np.float32)
    a_dst_heads = np.asarray(a_dst_heads, np.float32)
    W_out = np.asarray(W_out, np.float32)
    a_src_out = np.asarray(a_src_out, np.float32)
    a_dst_out = np.asarray(a_dst_out, np.float32)

    # ---- host precompute (tiny): per-head Wh, f_src, f_dst ----
    Wh = np.einsum("nf,hfk->hnk", x, W_heads).astype(np.float32)      # [H,N,FH]
    fsrc = np.einsum("hnk,hk->hn", Wh, a_src_heads).astype(np.float32)  # [H,N]
    fdst = np.einsum("hnk,hk->hn", Wh, a_dst_heads).astype(np.float32)  # [H,N]
    whaug = np.concatenate([Wh, np.ones((H, N, 1), np.float32)], axis=2)  # [H,N,FH+1]
    import ml_dtypes
    whaug_packed = np.ascontiguousarray(
        whaug.reshape(H, NJC, 128, FH + 1)
    ).astype(ml_dtypes.bfloat16)
    fdst_packed = np.ascontiguousarray(
        fdst.reshape(H, NJC, 128).transpose(2, 0, 1).reshape(128, H * NJC)
    )
    wout_packed = np.ascontiguousarray(W_out.reshape(8, 128, C)).astype(ml_dtypes.bfloat16)
    asrc_col = np.ascontiguousarray(a_src_out.reshape(C, 1))
    adst_row = np.ascontiguousarray(a_dst_out.reshape(1, C))

    # ea transposed: eaT[j, p*ISLAB + il] = edge_attr[p, i0+il, j]
    ea_t_full = np.ascontiguousarray(edge_attr.transpose(2, 0, 1))  # [N(j), P, N(i)]

    in_maps = []
    for c in range(NCORES):
        i0 = c * ISLAB
        in_maps.append({
            "ea": np.ascontiguousarray(
                ea_t_full[:, :, i0 : i0 + ISLAB].reshape(N, PI)
            ),
            "fsrc": np.ascontiguousarray(fsrc[:, i0 : i0 + ISLAB]),
            "fdst": fdst_packed,
            "whaug": whaug_packed,
            "wout": wout_packed,
            "asrc": asrc_col,
            "adst": adst_row,
        })
    return in_maps


def host_tail(logits):
    """elu + log_softmax on [N, C] logits."""
    l64 = logits.astype(np.float64)
    e = np.where(l64 > 0, l64, np.expm1(l64))
    m = e.max(axis=1, keepdims=True)
    ls = e - (m + np.log(np.exp(e - m).sum(axis=1, keepdims=True)))
    return ls.astype(np.float32)


def kernel(**inputs):
    in_maps = prepare_in_maps(**inputs)
    nc = _get_nc()
    res = run_bass_kernel_spmd(nc, in_maps, list(range(NCORES)), trace=TRACE)
    _LAST["res"] = res
    _LAST["exec_time_ns"] = res.exec_time_ns

    logits = np.empty((N, C), np.float32)
    for c in range(NCORES):
        i0 = c * ISLAB
        logits[i0 : i0 + ISLAB, :] = res.results[c]["out"].T
    return host_tail(logits)


# revision 6
# speedup vs baseline: 1.0307x; 1.0307x over previous
"""EGAT (edge-featured GAT) Trainium2 Bass kernel, 8-core SPMD.

Strategy: 1D node partition. Each core owns a 256-row slab of the N=2048
nodes. All [P,N,N] attention tensors live in SBUF transposed ([j, (p,i)]
layout, partition = neighbor j) so the attention*V contraction over j maps
directly onto the PE array. Attention state never touches DRAM between the
5 layers. The only cross-core exchange is an AllGather of the final layer's
Wh_out ([2048,17] incl. a ones column used to get softmax row sums for free
from the matmul).

Host side: Wh/f_src/f_dst for heads 1-4 depend only on inputs -> numpy.
Final elu+log_softmax on [2048,16] logits -> numpy.
"""

import sys
import os

sys.path.insert(0, "/opt/trn_rl_repo")

import numpy as np

import concourse.bass as bass
import concourse.tile as tile
from concourse import mybir
from concourse.bass_utils import run_bass_kernel_spmd
from concourse.masks import make_identity

# problem constants (hardcoded per contract)
N = 2048
P = 4
FIN = 256
FH = 64
H = 4
C = 16
ALPHA = 0.2
NCORES = 8
ISLAB = N // NCORES          # 256 rows per core
NJC = N // 128               # 16 j-chunks of 128 partitions
PI = P * ISLAB               # 1024 free elements per (p,i) tile

FP32 = mybir.dt.float32
BF16 = mybir.dt.bfloat16

TRACE = False                # test.py flips this for profiling
_LAST = {}                   # exec stats for test.py


def _bcast_ap(src_ap, nparts):
    """Partition-broadcast a [1, F] DRAM AP to [nparts, F]."""
    return bass.AP(
        tensor=src_ap.tensor,
        offset=src_ap.offset,
        ap=[[0, nparts]] + [list(d) for d in src_ap.ap[-1:]],
    )


def _split_multi_waits(nc):
    """walrus in this env accepts one sync-wait per compute instruction;
    split extras onto same-engine NoOps placed just before."""
    n = 0
    for fn in nc.m.functions:
        for bb in fn.blocks:
            new_list = []
            for inst in bb.instructions:
                si = inst.sync_info
                if si and si.on_wait and len(si.on_wait) > 1:
                    waits = list(si.on_wait)
                    for w in waits[:-1]:
                        new_list.append(
                            mybir.InstNoOp(
                                name=f"{inst.name}-wsplit{n}",
                                engine=inst.engine,
                                sync_info=mybir.SyncInfo(on_wait=[w], on_update=[]),
                            )
                        )
                        n += 1
                    inst.sync_info = mybir.SyncInfo(
                        on_wait=[waits[-1]], on_update=list(si.on_update or [])
                    )
                new_list.append(inst)
            bb.instructions = new_list
    return n


def _build_nc(reps=1):
    nc = bass.Bass(num_devices=NCORES)

    ea_p = nc.declare_dram_parameter("ea", [N, PI], FP32, isOutput=False)
    fsrc_p = nc.declare_dram_parameter("fsrc", [H, ISLAB], FP32, isOutput=False)
    fdst_p = nc.declare_dram_parameter("fdst", [128, H * NJC], FP32, isOutput=False)
    whaug_p = nc.declare_dram_parameter("whaug", [H, NJC, 128, FH + 1], BF16, isOutput=False)
    wout_p = nc.declare_dram_parameter("wout", [8, 128, C], BF16, isOutput=False)
    asrc_p = nc.declare_dram_parameter("asrc", [C, 1], FP32, isOutput=False)
    adst_p = nc.declare_dram_parameter("adst", [1, C], FP32, isOutput=False)
    out_p = nc.declare_dram_parameter("out", [C, ISLAB], FP32, isOutput=True)

    Act = mybir.ActivationFunctionType
    Alu = mybir.AluOpType

    with tile.TileContext(nc) as tc:
      import contextlib
      for _rep in range(reps):
        with contextlib.ExitStack() as ctx:
            singles = ctx.enter_context(tc.tile_pool(name="singles", bufs=1))
            dram = ctx.enter_context(tc.tile_pool(name="dram", bufs=1, space="DRAM"))
            fsrcbc_pool = ctx.enter_context(tc.tile_pool(name="fsrcbc", bufs=2))
            whaug_pool = ctx.enter_context(tc.tile_pool(name="whaug", bufs=2))
            ea_pool = ctx.enter_context(tc.tile_pool(name="ea", bufs=3))
            e_pool = ctx.enter_context(tc.tile_pool(name="e", bufs=3))
            eh_pool = ctx.enter_context(tc.tile_pool(name="eh", bufs=2))
            sc_pool = ctx.enter_context(tc.tile_pool(name="sc", bufs=2))
            u_pool = ctx.enter_context(tc.tile_pool(name="u", bufs=2))
            rrow_pool = ctx.enter_context(tc.tile_pool(name="rrow", bufs=2))
            rbc_pool = ctx.enter_context(tc.tile_pool(name="rbc", bufs=2))
            post_pool = ctx.enter_context(tc.tile_pool(name="post", bufs=4))
            av_psum = ctx.enter_context(tc.tile_pool(name="av", bufs=1, space="PSUM"))
            aux_psum = ctx.enter_context(tc.tile_pool(name="aux", bufs=1, space="PSUM"))
            tp_psum = ctx.enter_context(tc.tile_pool(name="tp", bufs=2, space="PSUM"))

            # ---- singles ----
            fdst_sb = singles.tile([128, H * NJC], FP32)
            nc.sync.dma_start(out=fdst_sb, in_=fdst_p[:, :])
            asrc_sb = singles.tile([C, 1], FP32)
            nc.sync.dma_start(out=asrc_sb, in_=asrc_p[:, :])
            adst_bc = singles.tile([128, C], FP32)
            nc.sync.dma_start(out=adst_bc, in_=_bcast_ap(adst_p[0:1, :], 128))
            identity = singles.tile([128, 128], FP32)
            make_identity(nc, identity)
            wout_sb = []
            for c8 in range(8):
                w = singles.tile([128, C], BF16, tag=f"wout{c8}", name=f"wout{c8}")
                nc.sync.dma_start(out=w, in_=wout_p[c8, :, :])
                wout_sb.append(w)
            xcatT = []
            for c8 in range(8):
                x = singles.tile([128, ISLAB], BF16, tag=f"xcat{c8}", name=f"xcat{c8}")
                xcatT.append(x)

            u_prev = [None] * NJC
            recip_bc_prev = None

            # ---------------- heads 1..4 ----------------
            for h in range(H):
                fsrc_bc = fsrcbc_pool.tile([128, ISLAB], FP32, tag="fsrcbc")
                nc.sync.dma_start(out=fsrc_bc, in_=_bcast_ap(fsrc_p[h : h + 1, :], 128))
                whaug_sb = []
                for jc in range(NJC):
                    w = whaug_pool.tile([128, FH + 1], BF16, tag=f"whaug{jc}", name=f"whaug{jc}")
                    nc.sync.dma_start(out=w, in_=whaug_p[h, jc, :, :])
                    whaug_sb.append(w)

                av = [av_psum.tile([FH + 1, ISLAB], FP32, tag=f"av{p}", name=f"av{p}") for p in range(P)]

                u_cur = [None] * NJC
                for jc in range(NJC):
                    e_t = e_pool.tile([128, ISLAB], FP32, tag="e")
                    idx = h * NJC + jc
                    nc.scalar.activation(
                        e_t, fsrc_bc, Act.Prelu,
                        bias=fdst_sb[:, idx : idx + 1], alpha=ALPHA,
                    )
                    sc_t = sc_pool.tile([128, PI], FP32, tag="sc")
                    if h == 0:
                        ea_t = ea_pool.tile([128, PI], FP32, tag="ea")
                        nc.sync.dma_start(
                            out=ea_t, in_=ea_p[jc * 128 : (jc + 1) * 128, :]
                        )
                        for p in range(P):
                            sl = slice(p * ISLAB, (p + 1) * ISLAB)
                            nc.vector.tensor_mul(sc_t[:, sl], e_t, ea_t[:, sl])
                    else:
                        eh_t = eh_pool.tile([128, PI], FP32, tag="eh")
                        for p in range(P):
                            sl = slice(p * ISLAB, (p + 1) * ISLAB)
                            nc.vector.tensor_mul(eh_t[:, sl], e_t, recip_bc_prev[:, sl])
                        for p in range(P):
                            sl = slice(p * ISLAB, (p + 1) * ISLAB)
                            nc.vector.tensor_mul(sc_t[:, sl], eh_t[:, sl], u_prev[jc][:, sl])
                    u_t = u_pool.tile([128, PI], BF16, tag=f"u{jc}")
                    nc.scalar.activation(u_t, sc_t, Act.Exp)
                    u_cur[jc] = u_t
                    for p in range(P):
                        sl = slice(p * ISLAB, (p + 1) * ISLAB)
                        nc.tensor.matmul(
                            av[p][:, :], whaug_sb[jc], u_t[:, sl],
                            start=(jc == 0), stop=(jc == NJC - 1),
                        )

                # ---- layer post: recip of row sums, xcat = elu(h' / s) ----
                recip_row = rrow_pool.tile([1, PI], FP32, tag="rrow")
                for p in range(P):
                    sl = slice(p * ISLAB, (p + 1) * ISLAB)
                    nc.vector.reciprocal(recip_row[:, sl], av[p][FH : FH + 1, :])
                rb = dram.tile([1, PI], FP32, tag=f"rb{h}")
                nc.sync.dma_start(out=rb[:], in_=recip_row)
                recip_bc = rbc_pool.tile([128, PI], FP32, tag="rbc")
                nc.sync.dma_start(out=recip_bc, in_=_bcast_ap(rb[0:1, :], 128))

                for p in range(P):
                    sl = slice(p * ISLAB, (p + 1) * ISLAB)
                    xn = post_pool.tile([FH, ISLAB], FP32, tag="xn", bufs=2)
                    nc.vector.tensor_mul(xn, av[p][0:FH, :], recip_bc[0:FH, sl])
                    m = post_pool.tile([FH, ISLAB], FP32, tag="m", bufs=2)
                    nc.vector.tensor_scalar_min(m, xn, 0.0)
                    g = post_pool.tile([FH, ISLAB], FP32, tag="g", bufs=2)
                    nc.scalar.activation(g, m, Act.Exp)
                    g1 = post_pool.tile([FH, ISLAB], FP32, tag="g1", bufs=2)
                    nc.vector.tensor_scalar_add(g1, g, -1.0)
                    cidx = h * 2 + p // 2
                    r0 = (p % 2) * FH
                    nc.vector.tensor_max(xcatT[cidx][r0 : r0 + FH, :], xn, g1)

                u_prev = u_cur
                recip_bc_prev = recip_bc

            # ---------------- final layer prep ----------------
            wo_ps = aux_psum.tile([C, ISLAB], FP32, tag="wo")
            for c8 in range(8):
                nc.tensor.matmul(
                    wo_ps[:, :], wout_sb[c8], xcatT[c8],
                    start=(c8 == 0), stop=(c8 == 7),
                )
            whoutT_sb = singles.tile([C, ISLAB], FP32, tag="whoutT")
            nc.vector.tensor_copy(whoutT_sb, wo_ps)

            fs5_ps = aux_psum.tile([1, ISLAB], FP32, tag="fs5")
            nc.tensor.matmul(fs5_ps[:, :], asrc_sb, whoutT_sb, start=True, stop=True)
            fs5_row = singles.tile([1, ISLAB], FP32, tag="fs5row")
            nc.vector.tensor_copy(fs5_row, fs5_ps)
            fs5_d = dram.tile([1, ISLAB], FP32, tag="fs5d")
            nc.sync.dma_start(out=fs5_d[:], in_=fs5_row)
            fsrc5_bc = singles.tile([128, ISLAB], FP32, tag="fsrc5bc")
            nc.sync.dma_start(out=fsrc5_bc, in_=_bcast_ap(fs5_d[0:1, :], 128))

            # transpose Wh_outT -> [i, c] staging with ones column, allgather
            ag_in = dram.tile([ISLAB, C], FP32, tag="agin")
            for half in range(2):
                tp = tp_psum.tile([128, C], FP32, tag="tp")
                nc.tensor.transpose(
                    tp, whoutT_sb[:, half * 128 : (half + 1) * 128],
                    identity[0:C, 0:C],
                )
                st = post_pool.tile([128, C], FP32, tag="st", bufs=2)
                nc.vector.tensor_copy(st, tp)
                nc.gpsimd.dma_start(
                    out=ag_in[half * 128 : (half + 1) * 128, :], in_=st
                )
            ag_out = dram.tile([N, C], FP32, tag="agout")
            nc.gpsimd.collective_compute(
                "AllGather", Alu.bypass,
                replica_groups=[list(range(NCORES))],
                ins=[ag_in.opt()], outs=[ag_out.opt()],
            )
            lhsT5f = singles.tile([128, NJC, C], FP32, tag="lhsT5f")
            nc.gpsimd.dma_start(
                out=lhsT5f,
                in_=ag_out[:, :].rearrange("(jc jp) c -> jp jc c", jp=128),
            )
            lhsT5 = singles.tile([128, NJC, FH + 1], BF16, tag="lhsT5")
            nc.vector.memset(lhsT5, 0.0)
            nc.vector.tensor_copy(lhsT5[:, :, 0:C], lhsT5f)
            nc.vector.memset(lhsT5[:, :, FH : FH + 1], 1.0)
            fdst5_sb = singles.tile([128, NJC], FP32, tag="fdst5")
            for jc in range(NJC):
                tt = post_pool.tile([128, C], FP32, tag="f5t", bufs=2)
                nc.vector.tensor_mul(tt, lhsT5f[:, jc, 0:C], adst_bc)
                nc.vector.tensor_reduce(
                    fdst5_sb[:, jc : jc + 1], tt, axis=mybir.AxisListType.X, op=Alu.add
                )

            # ---------------- final layer ----------------
            av5 = [av_psum.tile([FH + 1, ISLAB], FP32, tag=f"av{p}", name=f"av5{p}") for p in range(P)]
            for jc in range(NJC):
                e_t = e_pool.tile([128, ISLAB], FP32, tag="e")
                nc.scalar.activation(
                    e_t, fsrc5_bc, Act.Prelu,
                    bias=fdst5_sb[:, jc : jc + 1], alpha=ALPHA,
                )
                eh_t = eh_pool.tile([128, PI], FP32, tag="eh")
                sc_t = sc_pool.tile([128, PI], FP32, tag="sc")
                for p in range(P):
                    sl = slice(p * ISLAB, (p + 1) * ISLAB)
                    nc.vector.tensor_mul(eh_t[:, sl], e_t, recip_bc_prev[:, sl])
                for p in range(P):
                    sl = slice(p * ISLAB, (p + 1) * ISLAB)
                    nc.vector.tensor_mul(sc_t[:, sl], eh_t[:, sl], u_prev[jc][:, sl])
                u_t = u_pool.tile([128, PI], BF16, tag=f"u{jc}")
                nc.scalar.activation(u_t, sc_t, Act.Exp)
                for p in range(P):
                    sl = slice(p * ISLAB, (p + 1) * ISLAB)
                    nc.tensor.matmul(
                        av5[p][:, :], lhsT5[:, jc, :], u_t[:, sl],
                        start=(jc == 0), stop=(jc == NJC - 1),
                    )

            r5 = rrow_pool.tile([1, PI], FP32, tag="rrow")
            for p in range(P):
                sl = slice(p * ISLAB, (p + 1) * ISLAB)
                nc.vector.reciprocal(r5[:, sl], av5[p][FH : FH + 1, :])
            r5s = rrow_pool.tile([1, PI], FP32, tag="r5s")
            nc.vector.tensor_scalar_mul(r5s, r5, 1.0 / P)
            r5d = dram.tile([1, PI], FP32, tag="r5d")
            nc.sync.dma_start(out=r5d[:], in_=r5s)
            r5bc = rbc_pool.tile([128, PI], FP32, tag="rbc")
            nc.sync.dma_start(out=r5bc, in_=_bcast_ap(r5d[0:1, :], 128))

            acc = None
            for p in range(P):
                sl = slice(p * ISLAB, (p + 1) * ISLAB)
                t5 = post_pool.tile([C, ISLAB], FP32, tag=f"t5_{p}", bufs=1, name=f"t5_{p}")
                nc.vector.tensor_mul(t5, av5[p][0:C, :], r5bc[0:C, sl])
                if acc is None:
                    acc = t5
                else:
                    a2 = post_pool.tile([C, ISLAB], FP32, tag=f"acc{p}", bufs=1, name=f"acc{p}")
                    nc.vector.tensor_add(a2, acc, t5)
                    acc = a2
            nc.sync.dma_start(out=out_p[:, :], in_=acc)

    _split_multi_waits(nc)
    return nc


_NC_CACHE = None


def _get_nc():
    global _NC_CACHE
    if _NC_CACHE is None:
        _NC_CACHE = _build_nc(int(os.environ.get("EGAT_REPS", "1")))
    return _NC_CACHE


def prepare_in_maps(x, edge_attr, W_heads, a_src_heads, a_dst_heads, W_out, a_src_out, a_dst_out):
    x = np.asarray(x, np.float32)
    edge_attr = np.asarray(edge_attr, np.float32)
    W_heads = np.asarray(W_heads, np.float32)
    a_src_heads = np.asarray(a_src_heads, np.float32)
    a_dst_heads = np.asarray(a_dst_heads, np.float32)
    W_out = np.asarray(W_out, np.float32)
    a_src_out = np.asarray(a_src_out, np.float32)
    a_dst_out = np.asarray(a_dst_out, np.float32)

    # ---- host precompute (tiny): per-head Wh, f_src, f_dst ----
    Wh = np.einsum("nf,hfk->hnk", x, W_heads).astype(np.float32)      # [H,N,FH]
    fsrc = np.einsum("hnk,hk->hn", Wh, a_src_heads).astype(np.float32)  # [H,N]
    fdst = np.einsum("hnk,hk->hn", Wh, a_dst_heads).astype(np.float32)  # [H,N]
    whaug = np.concatenate([Wh, np.ones((H, N, 1), np.float32)], axis=2)  # [H,N,FH+1]
    import ml_dtypes
    whaug_packed = np.ascontiguousarray(
        whaug.reshape(H, NJC, 128, FH + 1)
    ).astype(ml_dtypes.bfloat16)
    fdst_packed = np.ascontiguousarray(
        fdst.reshape(H, NJC, 128).transpose(2, 0, 1).reshape(128, H * NJC)
    )
    wout_packed = np.ascontiguousarray(W_out.reshape(8, 128, C)).astype(ml_dtypes.bfloat16)
    asrc_col = np.ascontiguousarray(a_src_out.reshape(C, 1))
    adst_row = np.ascontiguousarray(a_dst_out.reshape(1, C))

    # ea transposed: eaT[j, p*ISLAB + il] = edge_attr[p, i0+il, j]
    ea_t_full = np.ascontiguousarray(edge_attr.transpose(2, 0, 1))  # [N(j), P, N(i)]

    in_maps = []
    for c in range(NCORES):
        i0 = c * ISLAB
        in_maps.append({
            "ea": np.ascontiguousarray(
                ea_t_full[:, :, i0 : i0 + ISLAB].reshape(N, PI)
            ),
            "fsrc": np.ascontiguousarray(fsrc[:, i0 : i0 + ISLAB]),
            "fdst": fdst_packed,
            "whaug": whaug_packed,
            "wout": wout_packed,
            "asrc": asrc_col,
            "adst": adst_row,
        })
    return in_maps


def host_tail(logits):
    """elu + log_softmax on [N, C] logits."""
    l64 = logits.astype(np.float64)
    e = np.where(l64 > 0, l64, np.expm1(l64))
    m = e.max(axis=1, keepdims=True)
    ls = e - (m + np.log(np.exp(e - m).sum(axis=1, keepdims=True)))
    return ls.astype(np.float32)


def kernel(**inputs):
    in_maps = prepare_in_maps(**inputs)
    nc = _get_nc()
    res = run_bass_kernel_spmd(nc, in_maps, list(range(NCORES)), trace=TRACE)
    _LAST["res"] = res
    _LAST["exec_time_ns"] = res.exec_time_ns

    logits = np.empty((N, C), np.float32)
    for c in range(NCORES):
        i0 = c * ISLAB
        logits[i0 : i0 + ISLAB, :] = res.results[c]["out"].T
    return host_tail(logits)


# revision 20
# speedup vs baseline: 420.1857x; 407.6509x over previous
"""EGAT (edge-featured GAT) Trainium2 Bass kernel, 8-core SPMD.

Strategy: 1D node partition. Each core owns a 256-row slab of the N=2048
nodes. All [P,N,N] attention tensors live in SBUF transposed ([j, (p,i)]
layout, partition = neighbor j) so the attention*V contraction over j maps
directly onto the PE array. Attention state never touches DRAM between the
5 layers. The only cross-core exchange is an AllGather of the final layer's
Wh_out ([2048,17] incl. a ones column used to get softmax row sums for free
from the matmul).

Host side: Wh/f_src/f_dst for heads 1-4 depend only on inputs -> numpy.
Final elu+log_softmax on [2048,16] logits -> numpy.
"""

import sys
import os

sys.path.insert(0, "/opt/trn_rl_repo")

import numpy as np

import concourse.bass as bass
import concourse.tile as tile
from concourse import mybir
from concourse.bass_utils import run_bass_kernel_spmd
from concourse.masks import make_identity

# problem constants (hardcoded per contract)
N = 2048
P = 4
FIN = 256
FH = 64
H = 4
C = 16
ALPHA = 0.2
NCORES = 8
ISLAB = N // NCORES          # 256 rows per core
NJC = N // 128               # 16 j-chunks of 128 partitions
PI = P * ISLAB               # 1024 free elements per (p,i) tile

FP32 = mybir.dt.float32
BF16 = mybir.dt.bfloat16

TRACE = False                # test.py flips this for profiling
_LAST = {}                   # exec stats for test.py


def _rep4_ap(t):
    """View a [128, ISLAB] tile as [128, P, ISLAB] with the free dim repeated
    P times (step-0 outer free loop)."""
    return bass.AP(tensor=t.tensor, offset=t.offset,
                   ap=[list(t.ap[0]), [0, P], list(t.ap[1])])


def _bcast_ap(src_ap, nparts):
    """Partition-broadcast a [1, F] DRAM AP to [nparts, F]."""
    return bass.AP(
        tensor=src_ap.tensor,
        offset=src_ap.offset,
        ap=[[0, nparts]] + [list(d) for d in src_ap.ap[-1:]],
    )


def _split_multi_waits(nc):
    """walrus in this env accepts one sync-wait per compute instruction;
    split extras onto same-engine NoOps placed just before."""
    n = 0
    for fn in nc.m.functions:
        for bb in fn.blocks:
            new_list = []
            for inst in bb.instructions:
                si = inst.sync_info
                if si and si.on_wait and len(si.on_wait) > 1:
                    waits = list(si.on_wait)
                    for w in waits[:-1]:
                        new_list.append(
                            mybir.InstNoOp(
                                name=f"{inst.name}-wsplit{n}",
                                engine=inst.engine,
                                sync_info=mybir.SyncInfo(on_wait=[w], on_update=[]),
                            )
                        )
                        n += 1
                    inst.sync_info = mybir.SyncInfo(
                        on_wait=[waits[-1]], on_update=list(si.on_update or [])
                    )
                new_list.append(inst)
            bb.instructions = new_list
    return n


def _build_nc(reps=1):
    nc = bass.Bass(num_devices=NCORES)

    ea_p = nc.declare_dram_parameter("ea", [N, PI], BF16, isOutput=False)
    fsrc_p = nc.declare_dram_parameter("fsrc", [H, ISLAB], FP32, isOutput=False)
    fdst_p = nc.declare_dram_parameter("fdst", [128, H * NJC], FP32, isOutput=False)
    whaug_p = nc.declare_dram_parameter("whaug", [H, NJC, 128, FH + 1], BF16, isOutput=False)
    wout_p = nc.declare_dram_parameter("wout", [8, 128, C], BF16, isOutput=False)
    asrc_p = nc.declare_dram_parameter("asrc", [C, 1], FP32, isOutput=False)
    adst_p = nc.declare_dram_parameter("adst", [1, C], FP32, isOutput=False)
    out_p = nc.declare_dram_parameter("out", [C, ISLAB], FP32, isOutput=True)

    Act = mybir.ActivationFunctionType
    Alu = mybir.AluOpType

    with tile.TileContext(nc) as tc:
      import contextlib
      for _rep in range(reps):
        with contextlib.ExitStack() as ctx:
            singles = ctx.enter_context(tc.tile_pool(name="singles", bufs=1))
            dram = ctx.enter_context(tc.tile_pool(name="dram", bufs=1, space="DRAM"))
            fsrcbc_pool = ctx.enter_context(tc.tile_pool(name="fsrcbc", bufs=2))
            whaug_pool = ctx.enter_context(tc.tile_pool(name="whaug", bufs=2))
            ea_pool = ctx.enter_context(tc.tile_pool(name="ea", bufs=12))
            e_pool = ctx.enter_context(tc.tile_pool(name="e", bufs=8))
            eh_pool = ctx.enter_context(tc.tile_pool(name="eh", bufs=3))
            sc_pool = ctx.enter_context(tc.tile_pool(name="sc", bufs=3))
            u_pool = ctx.enter_context(tc.tile_pool(name="u", bufs=2))
            rrow_pool = ctx.enter_context(tc.tile_pool(name="rrow", bufs=2))
            rbc_pool = ctx.enter_context(tc.tile_pool(name="rbc", bufs=2))
            post_pool = ctx.enter_context(tc.tile_pool(name="post", bufs=4))
            av_psum = ctx.enter_context(tc.tile_pool(name="av", bufs=1, space="PSUM"))


            # ---- small critical tiles first (they gate layer-1 startup) ----
            fdst_sb = singles.tile([128, H * NJC], FP32)
            nc.sync.dma_start(out=fdst_sb, in_=fdst_p[:, :])
            fsrc_bcs = []
            for h in range(H):
                fb = fsrcbc_pool.tile([128, ISLAB], FP32, tag=f"fsrcbc{h}", bufs=1, name=f"fsrcbc{h}")
                nc.sync.dma_start(out=fb, in_=_bcast_ap(fsrc_p[h : h + 1, :], 128))
                fsrc_bcs.append(fb)
            # edge slab prefetch: L1 is paced by its arrival
            ea_tiles = []
            for jc in range(NJC):
                ea_t = ea_pool.tile([128, PI], BF16, tag="ea", name=f"ea{jc}")
                nc.sync.dma_start(out=ea_t, in_=ea_p[jc * 128 : (jc + 1) * 128, :])
                ea_tiles.append(ea_t)
            asrc_sb = singles.tile([C, 1], FP32)
            nc.sync.dma_start(out=asrc_sb, in_=asrc_p[:, :])
            adst_bc = singles.tile([128, C], FP32)
            nc.sync.dma_start(out=adst_bc, in_=_bcast_ap(adst_p[0:1, :], 128))
            asrc2_sb = singles.tile([C, 1], FP32, tag="adstc2")
            nc.sync.dma_start(out=asrc2_sb, in_=adst_p[0:1, :].rearrange("a b -> b a"))
            identity = singles.tile([128, 128], FP32)
            make_identity(nc, identity)
            ones_bf = singles.tile([1, 128], BF16)
            nc.vector.memset(ones_bf, 1.0)
            wout_sb = []
            for c8 in range(8):
                w = singles.tile([128, C], BF16, tag=f"wout{c8}", name=f"wout{c8}")
                nc.sync.dma_start(out=w, in_=wout_p[c8, :, :])
                wout_sb.append(w)
            xcatT = []
            for c8 in range(8):
                x = singles.tile([128, ISLAB], BF16, tag=f"xcat{c8}", name=f"xcat{c8}")
                xcatT.append(x)

            u_prev = [None] * NJC
            recip_bc_prev = None

            # ---------------- heads 1..4 ----------------
            for h in range(H):
                fsrc_bc = fsrc_bcs[h]
                whaug_sb = []
                for jc in range(NJC):
                    w = whaug_pool.tile([128, FH + 1], BF16, tag=f"whaug{jc}", name=f"whaug{jc}")
                    nc.sync.dma_start(out=w, in_=whaug_p[h, jc, :, :])
                    whaug_sb.append(w)

                av = [av_psum.tile([FH + 1, ISLAB], FP32, tag=f"av{p}", name=f"av{p}") for p in range(P)]

                u_cur = [None] * NJC
                for jc in range(NJC):
                    idx = h * NJC + jc
                    e_t = e_pool.tile([128, ISLAB], BF16, tag="e")
                    nc.scalar.activation(
                        e_t, fsrc_bc, Act.Prelu,
                        bias=fdst_sb[:, idx : idx + 1], alpha=ALPHA,
                    )
                    sc_t = sc_pool.tile([128, PI], BF16, tag="sc")
                    if h == 0:
                        ea_t = ea_tiles[jc]
                        nc.vector.tensor_mul(
                            sc_t.rearrange("a (p i) -> a p i", p=P),
                            _rep4_ap(e_t), ea_t.rearrange("a (p i) -> a p i", p=P))
                    else:
                        sc2_t = eh_pool.tile([128, PI], BF16, tag="eh")
                        nc.vector.tensor_mul(
                            sc2_t.rearrange("a (p i) -> a p i", p=P),
                            _rep4_ap(e_t), u_prev[jc].rearrange("a (p i) -> a p i", p=P))
                        nc.vector.tensor_mul(sc_t, sc2_t, recip_bc_prev)
                    u_t = u_pool.tile([128, PI], BF16, tag=f"u{jc}")
                    nc.scalar.activation(u_t, sc_t, Act.Exp)
                    u_cur[jc] = u_t
                    for p in range(P):
                        sl = slice(p * ISLAB, (p + 1) * ISLAB)
                        nc.tensor.matmul(
                            av[p][:, :], whaug_sb[jc], u_t[:, sl],
                            start=(jc == 0), stop=(jc == NJC - 1),
                        )

                # ---- layer post: recip of row sums, xcat = elu(h' / s) ----
                recip_row = rrow_pool.tile([1, PI], FP32, tag="rrow")
                for p in range(P):
                    sl = slice(p * ISLAB, (p + 1) * ISLAB)
                    nc.vector.reciprocal(recip_row[:, sl], av[p][FH : FH + 1, :])
                rrow_bf = rrow_pool.tile([1, PI], BF16, tag="rrowbf")
                nc.vector.tensor_copy(rrow_bf, recip_row)
                recip_bc = rbc_pool.tile([128, PI], BF16, tag="rbc")
                for k in range(2):
                    rb_ps = av_psum.tile([128, PI // 2], FP32, tag="rbcps", bufs=2, name="rb_ps")
                    nc.tensor.matmul(rb_ps[:, :], ones_bf,
                                     rrow_bf[:, k * (PI // 2) : (k + 1) * (PI // 2)],
                                     start=True, stop=True)
                    nc.vector.tensor_copy(recip_bc[:, k * (PI // 2) : (k + 1) * (PI // 2)], rb_ps)

                xn = post_pool.tile([FH, PI], BF16, tag="xn", bufs=2)
                for p in range(P):
                    sl = slice(p * ISLAB, (p + 1) * ISLAB)
                    nc.vector.tensor_mul(xn[:, sl], av[p][0:FH, :], recip_bc[0:FH, sl])
                m = post_pool.tile([FH, PI], BF16, tag="m", bufs=1)
                nc.vector.tensor_scalar_min(m, xn, 0.0)
                g = post_pool.tile([FH, PI], BF16, tag="g", bufs=1)
                nc.scalar.activation(g, m, Act.Exp)
                g1 = post_pool.tile([FH, PI], BF16, tag="g1", bufs=1)
                nc.vector.tensor_scalar_add(g1, g, -1.0)
                for p in range(P):
                    sl = slice(p * ISLAB, (p + 1) * ISLAB)
                    cidx = h * 2 + p // 2
                    r0 = (p % 2) * FH
                    nc.vector.tensor_max(xcatT[cidx][r0 : r0 + FH, :], xn[:, sl], g1[:, sl])

                u_prev = u_cur
                recip_bc_prev = recip_bc

            # ---------------- final layer prep ----------------
            wo_ps = av_psum.tile([C, ISLAB], FP32, tag="av0", bufs=1, name="wo_ps")
            for c8 in range(8):
                nc.tensor.matmul(
                    wo_ps[:, :], wout_sb[c8], xcatT[c8],
                    start=(c8 == 0), stop=(c8 == 7),
                )
            whoutT_sb = singles.tile([C, ISLAB], FP32, tag="whoutT")
            nc.vector.tensor_copy(whoutT_sb, wo_ps)

            fs5_ps = av_psum.tile([1, ISLAB], FP32, tag="av1", bufs=1, name="fs5_ps")
            nc.tensor.matmul(fs5_ps[:, :], asrc_sb, whoutT_sb, start=True, stop=True)
            fs5_row = singles.tile([1, ISLAB], BF16, tag="fs5row")
            nc.vector.tensor_copy(fs5_row, fs5_ps)
            fsrc5_bc = singles.tile([128, ISLAB], FP32, tag="fsrc5bc")
            fs5b_ps = av_psum.tile([128, ISLAB], FP32, tag="rbcps", bufs=2, name="fs5b_ps")
            nc.tensor.matmul(fs5b_ps[:, :], ones_bf, fs5_row, start=True, stop=True)
            nc.scalar.copy(fsrc5_bc, fs5b_ps)

            # transpose Wh_outT -> [i, c] staging with ones column, allgather
            ag_in = dram.tile([ISLAB, C + 1], BF16, tag="agin")
            for half in range(2):
                tp = av_psum.tile([128, C], FP32, tag="av2", bufs=1, name="tp")
                nc.tensor.transpose(
                    tp, whoutT_sb[:, half * 128 : (half + 1) * 128],
                    identity[0:C, 0:C],
                )
                fd_ps = av_psum.tile([128, 1], FP32, tag="av1", bufs=1, name="fd_ps")
                nc.tensor.matmul(fd_ps[:, :],
                                 whoutT_sb[:, half * 128 : (half + 1) * 128],
                                 asrc2_sb, start=True, stop=True)
                st = post_pool.tile([128, C + 1], BF16, tag="st", bufs=2)
                nc.vector.tensor_copy(st[:, 0:C], tp)
                nc.vector.tensor_copy(st[:, C : C + 1], fd_ps)
                nc.gpsimd.dma_start(
                    out=ag_in[half * 128 : (half + 1) * 128, :], in_=st
                )
            ag_out = dram.tile([N, C + 1], BF16, tag="agout")
            nc.gpsimd.collective_compute(
                "AllGather", Alu.bypass,
                replica_groups=[list(range(NCORES))],
                ins=[ag_in.opt()], outs=[ag_out.opt()],
            )
            lhsT5f = singles.tile([128, NJC, C + 1], BF16, tag="lhsT5f")
            nc.gpsimd.dma_start(
                out=lhsT5f,
                in_=ag_out[:, :].rearrange("(jc jp) c -> jp jc c", jp=128),
            )
            lhsT5 = singles.tile([128, NJC, FH + 1], BF16, tag="lhsT5")
            nc.vector.memset(lhsT5, 0.0)
            nc.vector.tensor_copy(lhsT5[:, :, 0:C], lhsT5f[:, :, 0:C])
            nc.vector.memset(lhsT5[:, :, FH : FH + 1], 1.0)
            fdst5_sb = singles.tile([128, NJC], FP32, tag="fdst5")
            nc.vector.tensor_copy(fdst5_sb, lhsT5f[:, :, C])

            # ---------------- final layer ----------------
            q5 = []
            for jc in range(NJC):
                q_t = u_pool.tile([128, PI], BF16, tag=f"q{jc}", bufs=1, name=f"q{jc}")
                nc.vector.tensor_mul(q_t, u_prev[jc], recip_bc_prev)
                q5.append(q_t)
            av5 = [av_psum.tile([FH + 1, ISLAB], FP32, tag=f"av{p}", name=f"av5{p}") for p in range(P)]
            for jc in range(NJC):
                e_t = e_pool.tile([128, ISLAB], BF16, tag="e")
                nc.scalar.activation(
                    e_t, fsrc5_bc, Act.Prelu,
                    bias=fdst5_sb[:, jc : jc + 1], alpha=ALPHA,
                )
                sc_t = sc_pool.tile([128, PI], BF16, tag="sc")
                nc.vector.tensor_mul(
                    sc_t.rearrange("a (p i) -> a p i", p=P),
                    _rep4_ap(e_t), q5[jc].rearrange("a (p i) -> a p i", p=P))
                u_t = u_pool.tile([128, PI], BF16, tag=f"u{jc}")
                nc.scalar.activation(u_t, sc_t, Act.Exp)
                for p in range(P):
                    sl = slice(p * ISLAB, (p + 1) * ISLAB)
                    nc.tensor.matmul(
                        av5[p][:, :], lhsT5[:, jc, :], u_t[:, sl],
                        start=(jc == 0), stop=(jc == NJC - 1),
                    )

            r5 = rrow_pool.tile([1, PI], FP32, tag="rrow")
            for p in range(P):
                sl = slice(p * ISLAB, (p + 1) * ISLAB)
                nc.vector.reciprocal(r5[:, sl], av5[p][FH : FH + 1, :])
            r5s = rrow_pool.tile([1, PI], BF16, tag="r5s")
            nc.vector.tensor_scalar_mul(r5s, r5, 1.0 / P)
            r5bc = rbc_pool.tile([128, PI], FP32, tag="rbc5")
            for k in range(2):
                rb_ps = av_psum.tile([128, PI // 2], FP32, tag="rbcps", bufs=2, name="rb_ps5")
                nc.tensor.matmul(rb_ps[:, :], ones_bf,
                                 r5s[:, k * (PI // 2) : (k + 1) * (PI // 2)],
                                 start=True, stop=True)
                nc.scalar.copy(r5bc[:, k * (PI // 2) : (k + 1) * (PI // 2)], rb_ps)

            acc = None
            for p in range(P):
                sl = slice(p * ISLAB, (p + 1) * ISLAB)
                t5 = post_pool.tile([C, ISLAB], FP32, tag=f"t5_{p}", bufs=1, name=f"t5_{p}")
                nc.vector.tensor_mul(t5, av5[p][0:C, :], r5bc[0:C, sl])
                if acc is None:
                    acc = t5
                else:
                    a2 = post_pool.tile([C, ISLAB], FP32, tag=f"acc{p}", bufs=1, name=f"acc{p}")
                    nc.vector.tensor_add(a2, acc, t5)
                    acc = a2
            nc.sync.dma_start(out=out_p[:, :], in_=acc)

    _split_multi_waits(nc)
    return nc


_NC_CACHE = None


def _get_nc():
    global _NC_CACHE
    if _NC_CACHE is None:
        _NC_CACHE = _build_nc(int(os.environ.get("EGAT_REPS", "1")))
    return _NC_CACHE


def prepare_in_maps(x, edge_attr, W_heads, a_src_heads, a_dst_heads, W_out, a_src_out, a_dst_out):
    x = np.asarray(x, np.float32)
    edge_attr = np.asarray(edge_attr, np.float32)
    W_heads = np.asarray(W_heads, np.float32)
    a_src_heads = np.asarray(a_src_heads, np.float32)
    a_dst_heads = np.asarray(a_dst_heads, np.float32)
    W_out = np.asarray(W_out, np.float32)
    a_src_out = np.asarray(a_src_out, np.float32)
    a_dst_out = np.asarray(a_dst_out, np.float32)

    import ml_dtypes
    # ---- host precompute (tiny): per-head Wh, f_src, f_dst ----
    Wh = np.einsum("nf,hfk->hnk", x, W_heads).astype(np.float32)      # [H,N,FH]
    fsrc = np.einsum("hnk,hk->hn", Wh, a_src_heads).astype(np.float32)  # [H,N]
    fdst = np.einsum("hnk,hk->hn", Wh, a_dst_heads).astype(np.float32)  # [H,N]
    whaug = np.concatenate([Wh, np.ones((H, N, 1), np.float32)], axis=2)  # [H,N,FH+1]
    whaug_packed = np.ascontiguousarray(
        whaug.reshape(H, NJC, 128, FH + 1)
    ).astype(ml_dtypes.bfloat16)
    fdst_packed = np.ascontiguousarray(
        fdst.reshape(H, NJC, 128).transpose(2, 0, 1).reshape(128, H * NJC)
    )
    wout_packed = np.ascontiguousarray(W_out.reshape(8, 128, C)).astype(ml_dtypes.bfloat16)
    asrc_col = np.ascontiguousarray(a_src_out.reshape(C, 1))
    adst_row = np.ascontiguousarray(a_dst_out.reshape(1, C))

    # ea transposed: eaT[j, p*ISLAB + il] = edge_attr[p, i0+il, j]
    ea_t_full = np.ascontiguousarray(edge_attr.transpose(2, 0, 1))  # [N(j), P, N(i)]

    in_maps = []
    for c in range(NCORES):
        i0 = c * ISLAB
        in_maps.append({
            "ea": np.ascontiguousarray(
                ea_t_full[:, :, i0 : i0 + ISLAB].reshape(N, PI)
            ).astype(ml_dtypes.bfloat16),
            "fsrc": np.ascontiguousarray(fsrc[:, i0 : i0 + ISLAB]),
            "fdst": fdst_packed,
            "whaug": whaug_packed,
            "wout": wout_packed,
            "asrc": asrc_col,
            "adst": adst_row,
        })
    return in_maps


def host_tail(logits):
    """elu + log_softmax on [N, C] logits."""
    l64 = logits.astype(np.float64)
    e = np.where(l64 > 0, l64, np.expm1(l64))
    m = e.max(axis=1, keepdims=True)
    ls = e - (m + np.log(np.exp(e - m).sum(axis=1, keepdims=True)))
    return ls.astype(np.float32)


def kernel(**inputs):
    in_maps = prepare_in_maps(**inputs)
    nc = _get_nc()
    res = run_bass_kernel_spmd(nc, in_maps, list(range(NCORES)), trace=TRACE)
    _LAST["res"] = res
    _LAST["exec_time_ns"] = res.exec_time_ns

    logits = np.empty((N, C), np.float32)
    for c in range(NCORES):
        i0 = c * ISLAB
        logits[i0 : i0 + ISLAB, :] = res.results[c]["out"].T
    return host_tail(logits)
